# revision 1
# baseline (speedup 1.0000x reference)
"""ConvBert LightConv kernel v4 for Trainium2 (Bass/Tile), batch-parallel on 8
cores. Fully fused: no DRAM scratch, no separate prepass.

out[b,s,h,c] = sum_j softmax_j(filters[b,s,h,:])[j] * x_pad[b, s+j-4, h*64+c]

Per-core, per main tile (TW=120 output tokens, 35 tiles):
  x_t  [128,768] f32 <- x rows [t0-4, t0+124)      (halo for the 9 taps)
  fr_t [128,108] f32 <- f rows [t0-8, t0+120)      (halo for the stagger)
  softmax in-tile: exp (ACT), reduce+recip (DVE), normalize -> bf16 [p,j,h]
    (Pool, transposing (h,j)->(j,h) via strided read).
  stagger via PE: for each tap j, matmul with lhsT = B[:, 16-j : 144-j]
    (column slice of one inline band matrix B[k,c]=[k==c-8], bf16) maps
    fnb[q] -> fn_ps[k] = fnb[k-j+8], i.e. fn_ps[k,j,h] = fn[t0+k-j,h,j] --
    the filter value multiplying x row t0-4+k toward output row t0+k-j.
    Rows where k-j+8 >= 128 get 0 (no identity hit), so no garbage.
  evac fn_stag [128,108] f32 <- fn_ps (DVE; Pool cannot read PSUM).
  products: P[k,j,hc] = x_t[k,hc] * fn_stag[k,j,h] (broadcast over c via
    step-0 AP), fp32 inputs -> bf16 out, split DVE/Pool.
  shift-sum: out[t0+m] = sum_j P[m+j,j] -- lhsT = ident[:, j:j+tw] column
    slices of one 128x128 bf16 identity, 9 accumulating PE matmuls into
    PSUM (exact: weights 0/1), in 512+256 column chunks.
  evac o_t (ACT copy), DMA out.
Zero padding at sequence edges via memset of x_t / fr_t edge rows
(exp(0)=1 keeps softmax finite; stagger zeros + x zeros keep PE NaN-free).
"""

import os
import sys

import numpy as np

for _p in ("/opt/trn_rl_repo",):
    if _p not in sys.path:
        sys.path.insert(0, _p)

B, S, D = 8, 4096, 768
H, HD, KS = 12, 64, 9
PAD = KS // 2  # 4
TW = 120  # output tokens per main tile
NT = (S + TW - 1) // TW  # 35 tiles; last covers 16 tokens

_CACHE = {}


def _build_program(loop_n: int | None = None):
    """loop_n=None: single-shot program (used by kernel()). loop_n=K: the
    whole kernel body runs K times under a tc.For_i hardware loop — one NEFF
    execution = K complete DRAM->DRAM kernel executions (bench)."""
    import concourse.bass as bass
    import concourse.tile as tile
    from concourse import mybir

    f32 = mybir.dt.float32
    bf16 = mybir.dt.bfloat16

    nc = bass.Bass()
    x_d = nc.dram_tensor("x", [S, D], f32, kind="ExternalInput")
    f_d = nc.dram_tensor("f", [S, H * KS], f32, kind="ExternalInput")
    o_d = nc.dram_tensor("o", [S, D], f32, kind="ExternalOutput")

    # ident[k,m] = [k==m]; shift-sum lhsT_j = ident[:, j : j+tw]
    id_np = np.eye(128, dtype=np.float32)
    id_d = nc.inline_tensor(id_np, name="ident")
    # band B[k,c] = [k == c-8]; stagger lhsT_j = B[:, 16-j : 144-j]
    b_np = np.zeros((128, 144), dtype=np.float32)
    for k in range(128):
        b_np[k, k + 8] = 1.0
    b_d = nc.inline_tensor(b_np, name="band")

    with tile.TileContext(nc) as tc:
        with (
            tc.tile_pool(name="singles", bufs=1) as singles,
            tc.tile_pool(name="fin", bufs=4) as fin,
            tc.tile_pool(name="xin", bufs=4) as xin,
            tc.tile_pool(name="sfx", bufs=4) as sfx,
            tc.tile_pool(name="fst", bufs=4) as fst,
            tc.tile_pool(name="prod", bufs=3) as prod,
            tc.tile_pool(name="prodb", bufs=3) as prodb,
            tc.tile_pool(name="outs", bufs=4) as outs,
            tc.tile_pool(name="psf", bufs=2, space="PSUM") as psf,
            tc.tile_pool(name="ps", bufs=3, space="PSUM") as ps,
        ):
            id_f32 = singles.tile([128, 128], f32)
            nc.sync.dma_start(out=id_f32, in_=id_d[:, :])
            id_sb = singles.tile([128, 128], bf16)
            nc.vector.tensor_copy(id_sb, id_f32)
            b_f32 = singles.tile([128, 144], f32)
            nc.sync.dma_start(out=b_f32, in_=b_d[:, :])
            b_sb = singles.tile([128, 144], bf16)
            nc.vector.tensor_copy(b_sb, b_f32)

            def _backhalf(t0, tw, p_t):
                o_ps = ps.tile([128, D], f32, tag="o_ps")
                for j in range(KS):
                    lhsT = id_sb[:, j : j + tw]
                    for n0, n1 in ((0, 512), (512, D)):
                        nc.tensor.matmul(
                            o_ps[0:tw, n0:n1],
                            lhsT,
                            p_t[:, j, n0:n1],
                            start=(j == 0),
                            stop=(j == KS - 1),
                        )
                o_t = outs.tile([128, D], f32, tag="o_t")
                nc.scalar.activation(
                    o_t[0:tw, :], o_ps[0:tw, :], mybir.ActivationFunctionType.Copy
                )
                nc.sync.dma_start(out=o_d[t0 : t0 + tw, :], in_=o_t[0:tw, :])

            def _kernel_body():
                for t in range(NT):
                    t0 = TW * t
                    tw = min(TW, S - t0)  # valid out tokens (16 on last tile)
                    u0 = t0 - PAD  # first x row of this u-tile
                    f0 = t0 - 2 * PAD  # first f row (stagger halo)

                    x_t = xin.tile([128, D], f32, tag="x_t")
                    if t == 0:
                        nc.vector.memset(x_t[0:PAD, :], 0.0)
                        nc.sync.dma_start(
                            out=x_t[PAD:128, :], in_=x_d[0 : 128 - PAD, :]
                        )
                    elif u0 + 128 > S:
                        nv = S - u0
                        nc.vector.memset(x_t, 0.0)
                        nc.sync.dma_start(out=x_t[0:nv, :], in_=x_d[u0:S, :])
                    else:
                        nc.sync.dma_start(out=x_t, in_=x_d[u0 : u0 + 128, :])

                    fr_t = fin.tile([128, H * KS], f32, tag="fr_t")
                    if t == 0:
                        nc.gpsimd.memset(fr_t[0 : 2 * PAD, :], 0.0)
                        nc.scalar.dma_start(
                            out=fr_t[2 * PAD : 128, :], in_=f_d[0 : 128 - 2 * PAD, :]
                        )
                    elif f0 + 128 > S:
                        nv = S - f0
                        nc.gpsimd.memset(fr_t, 0.0)
                        nc.scalar.dma_start(out=fr_t[0:nv, :], in_=f_d[f0:S, :])
                    else:
                        nc.scalar.dma_start(out=fr_t, in_=f_d[f0 : f0 + 128, :])

                    e_t = sfx.tile([128, H * KS], f32, tag="e_t")
                    nc.scalar.activation(
                        e_t, fr_t, mybir.ActivationFunctionType.Exp
                    )
                    z_t = sfx.tile([128, H], f32, tag="z_t")
                    nc.vector.tensor_reduce(
                        out=z_t,
                        in_=e_t.rearrange("p (h j) -> p h j", j=KS),
                        axis=mybir.AxisListType.X,
                        op=mybir.AluOpType.add,
                    )
                    r_t = sfx.tile([128, H], f32, tag="r_t")
                    nc.vector.reciprocal(r_t, z_t)
                    fnb = sfx.tile([128, KS, H], bf16, tag="fnb")
                    nc.gpsimd.tensor_mul(
                        fnb,
                        e_t.rearrange("p (h j) -> p j h", j=KS),
                        r_t.unsqueeze(1).broadcast_to([128, KS, H]),
                    )

                    fn_ps = psf.tile([128, KS, H], f32, tag="fn_ps")
                    for j in range(KS):
                        nc.tensor.matmul(
                            fn_ps[:, j, :],
                            b_sb[:, 16 - j : 144 - j],
                            fnb[:, j, :],
                            start=True,
                            stop=True,
                        )
                    fn_s = fst.tile([128, KS, H], f32, tag="fn_s")
                    nc.scalar.activation(
                        fn_s, fn_ps, mybir.ActivationFunctionType.Copy
                    )

                    # Products. HW-measured rates: DVE 1.04 ns/col (matches
                    # model), Pool (gpsimd) 1.95 ns/col (2.3x model) -> give
                    # DVE taps 0-5, Pool taps 6-8, one merged op per engine.
                    if os.environ.get("LC_V5_ALT"):
                        # Whole-tile alternation: one engine does all 9 taps
                        # of a tile; tiles 2,5,8,... go to Pool (rate-matched
                        # 24:11 split), so the engines never share operands.
                        eng = nc.gpsimd if (t % 3 == 2) else nc.vector
                        p_t = prod.tile([128, KS, D], bf16, tag="p_t")
                        x_hc = x_t.rearrange("p (h c) -> p h c", c=HD)
                        eng.tensor_mul(
                            p_t.rearrange("p j (h c) -> p j h c", c=HD),
                            x_hc.unsqueeze(1).broadcast_to([128, KS, H, HD]),
                            fn_s.unsqueeze(3).broadcast_to([128, KS, H, HD]),
                        )
                        _backhalf(t0, tw, p_t)
                        continue

                    if os.environ.get("LC_V5_SPLITP"):
                        # Separate tensors per engine: avoids any serialization
                        # from same-tensor concurrent writes.
                        NDV = 6
                        p_a = prod.tile([128, NDV, D], bf16, tag="p_a")
                        p_b = prodb.tile([128, KS - NDV, D], bf16, tag="p_b")
                        x_hc = x_t.rearrange("p (h c) -> p h c", c=HD)
                        nc.vector.tensor_mul(
                            p_a.rearrange("p j (h c) -> p j h c", c=HD),
                            x_hc.unsqueeze(1).broadcast_to([128, NDV, H, HD]),
                            fn_s[:, 0:NDV, :]
                            .unsqueeze(3)
                            .broadcast_to([128, NDV, H, HD]),
                        )
                        nc.gpsimd.tensor_mul(
                            p_b.rearrange("p j (h c) -> p j h c", c=HD),
                            x_hc.unsqueeze(1).broadcast_to([128, KS - NDV, H, HD]),
                            fn_s[:, NDV:KS, :]
                            .unsqueeze(3)
                            .broadcast_to([128, KS - NDV, H, HD]),
                        )

                        o_ps = ps.tile([128, D], f32, tag="o_ps")
                        for j in range(KS):
                            lhsT = id_sb[:, j : j + tw]
                            rhs_t = p_a if j < NDV else p_b
                            jj = j if j < NDV else j - NDV
                            for n0, n1 in ((0, 512), (512, D)):
                                nc.tensor.matmul(
                                    o_ps[0:tw, n0:n1],
                                    lhsT,
                                    rhs_t[:, jj, n0:n1],
                                    start=(j == 0),
                                    stop=(j == KS - 1),
                                )
                        o_t = outs.tile([128, D], f32, tag="o_t")
                        nc.scalar.activation(
                            o_t[0:tw, :],
                            o_ps[0:tw, :],
                            mybir.ActivationFunctionType.Copy,
                        )
                        nc.sync.dma_start(out=o_d[t0 : t0 + tw, :], in_=o_t[0:tw, :])
                        continue

                    p_t = prod.tile([128, KS, D], bf16, tag="p_t")
                    x_hc = x_t.rearrange("p (h c) -> p h c", c=HD)
                    if os.environ.get("LC_V5_PERTAP"):
                        for j in range(KS):
                            eng = nc.vector if j < 6 else nc.gpsimd
                            eng.tensor_mul(
                                p_t[:, j, :].rearrange("p (h c) -> p h c", c=HD),
                                x_hc,
                                fn_s[:, j, :].unsqueeze(2).broadcast_to([128, H, HD]),
                            )
                    else:
                        for eng, j0, j1 in ((nc.vector, 0, 6), (nc.gpsimd, 6, KS)):
                            nj = j1 - j0
                            eng.tensor_mul(
                                p_t[:, j0:j1, :].rearrange("p j (h c) -> p j h c", c=HD),
                                x_hc.unsqueeze(1).broadcast_to([128, nj, H, HD]),
                                fn_s[:, j0:j1, :]
                                .unsqueeze(3)
                                .broadcast_to([128, nj, H, HD]),
                            )
                    if os.environ.get("LC_V5_NOBACK"):
                        continue

                    o_ps = ps.tile([128, D], f32, tag="o_ps")
                    for j in range(KS):
                        lhsT = id_sb[:, j : j + tw]
                        for n0, n1 in ((0, 512), (512, D)):
                            nc.tensor.matmul(
                                o_ps[0:tw, n0:n1],
                                lhsT,
                                p_t[:, j, n0:n1],
                                start=(j == 0),
                                stop=(j == KS - 1),
                            )

                    o_t = outs.tile([128, D], f32, tag="o_t")
                    nc.scalar.activation(
                        o_t[0:tw, :], o_ps[0:tw, :], mybir.ActivationFunctionType.Copy
                    )
                    nc.sync.dma_start(out=o_d[t0 : t0 + tw, :], in_=o_t[0:tw, :])

            if loop_n is None:
                _kernel_body()
            else:
                with tc.For_i(0, loop_n):
                    _kernel_body()

    _split_hwdge_multi_waits(nc)
    return nc


def _split_hwdge_multi_waits(nc):
    """walrus's HWDGE DMA trigger (PSEUDO_DMA_DIRECT2D) rejects >1 sync wait
    on a DMACopy. Move all but one wait onto a NoOp inserted right before the
    DMA on the same (sequencer) engine — identical semantics, since the
    sequencer executes both in order before triggering the descriptor."""
    from concourse import mybir

    nsplit = 0
    for fn in nc.m.functions:
        for blk in fn.blocks:
            out = []
            for ins in blk.instructions:
                si = ins.sync_info
                if si is not None and len(si.on_wait) > 1:
                    for wi, w in enumerate(si.on_wait[:-1]):
                        nop = mybir.InstNoOp(
                            name=f"{ins.name}_waitsplit{wi}",
                            engine=ins.engine,
                            sync_info=mybir.SyncInfo(on_wait=[w], on_update=[]),
                        )
                        out.append(nop)
                    ins.sync_info = mybir.SyncInfo(
                        on_wait=list(si.on_wait[-1:]),
                        on_update=list(si.on_update),
                    )
                    nsplit += 1
                out.append(ins)
            blk.instructions = out
    if nsplit and os.environ.get("LC_DEBUG"):
        print(f"_split_hwdge_multi_waits: split {nsplit} DMAs")


def kernel(inputs: np.ndarray, filters: np.ndarray) -> np.ndarray:
    from concourse.bass_utils import run_bass_kernel_spmd

    if "nc" not in _CACHE:
        _CACHE["nc"] = _build_program()
    nc = _CACHE["nc"]

    inputs = np.ascontiguousarray(np.asarray(inputs, dtype=np.float32))
    filters = np.ascontiguousarray(np.asarray(filters, dtype=np.float32))

    in_maps = [{"x": inputs[c], "f": filters[c]} for c in range(B)]

    res = run_bass_kernel_spmd(nc, in_maps, core_ids=list(range(B)), trace=False)

    out = np.stack([res.results[c]["o"] for c in range(B)], axis=0)
    return out.reshape(B, S, H, HD)

def bench(
    inputs: np.ndarray, filters: np.ndarray, reps: int = 20, loop_n: int = 1000
) -> float:
    """Steady-state device benchmark. One NEFF launch executes the complete
    kernel (prepass + main pass, full DRAM->DRAM dataflow) ``loop_n`` times
    under a tc.For_i hardware loop; ``reps`` launches are timed back-to-back
    after a warm-up launch. Returns mean seconds per kernel execution —
    launch/transfer overheads are amortized over reps*loop_n executions."""
    import time

    import jax
    from jax.experimental.shard_map import shard_map
    from jax.sharding import Mesh, PartitionSpec

    import concourse.mybir as mybir
    from concourse import bass2jax

    key = f"nc_loop{loop_n}"
    if key not in _CACHE:
        _CACHE[key] = _build_program(loop_n=loop_n)
    nc = _CACHE[key]
    bass2jax.install_neuronx_cc_hook()

    part_name = nc.partition_id_tensor.name if nc.partition_id_tensor else None
    in_names, out_names, out_avals, zero_outs = [], [], [], []
    for alloc in nc.m.functions[0].allocations:
        if not isinstance(alloc, mybir.MemoryLocationSet):
            continue
        name = alloc.memorylocations[0].name
        if alloc.kind == "ExternalInput":
            if name != part_name:
                in_names.append(name)
        elif alloc.kind == "ExternalOutput":
            out_names.append(name)
            shape = tuple(alloc.tensor_shape)
            dtype = mybir.dt.np(alloc.dtype)
            out_avals.append(jax.core.ShapedArray(shape, dtype))
            zero_outs.append(np.zeros(shape, dtype))
    n_params = len(in_names)
    all_names = in_names + out_names
    if part_name is not None:
        all_names = all_names + [part_name]

    def _body(*args):
        operands = list(args)
        if part_name is not None:
            operands.append(bass2jax.partition_id_tensor())
        outs = bass2jax._bass_exec_p.bind(
            *operands,
            out_avals=tuple(out_avals),
            in_names=tuple(all_names),
            out_names=tuple(out_names),
            lowering_input_output_aliases=(),
            sim_require_finite=True,
            sim_require_nnan=True,
            nc=nc,
        )
        return tuple(outs)

    devices = jax.devices()[:B]
    mesh = Mesh(np.asarray(devices), ("core",))
    nin = n_params + len(out_names)
    fn = jax.jit(
        shard_map(
            _body,
            mesh=mesh,
            in_specs=(PartitionSpec("core"),) * nin,
            out_specs=(PartitionSpec("core"),) * len(out_names),
            check_rep=False,
        ),
        keep_unused=True,
    )
    per_core = {"x": inputs.astype(np.float32), "f": filters.astype(np.float32)}
    concat_in = [
        np.concatenate([per_core[n][c] for c in range(B)], axis=0) for n in in_names
    ]
    concat_zero = [
        np.zeros((B * z.shape[0], *z.shape[1:]), z.dtype) for z in zero_outs
    ]
    sharding = jax.sharding.NamedSharding(mesh, PartitionSpec("core"))
    dev_args = [jax.device_put(a, sharding) for a in concat_in + concat_zero]

    out = fn(*dev_args)  # compile + warm
    jax.block_until_ready(out)
    t0 = time.perf_counter()
    for _ in range(reps):
        out = fn(*dev_args)
    jax.block_until_ready(out)
    t1 = time.perf_counter()
    return (t1 - t0) / (reps * loop_n)


if __name__ == "__main__":
    rng = np.random.default_rng(0)
    x = rng.standard_normal((B, S, D), dtype=np.float32)
    f = rng.standard_normal((B, S, H * KS), dtype=np.float32)
    o = kernel(x, f)
    print(o.shape, o.dtype)



# revision 2
# speedup vs baseline: 1.0090x; 1.0090x over previous
"""ConvBert LightConv kernel v7 for Trainium2 (Bass/Tile), batch-parallel on 8
cores. Fully fused: no DRAM scratch, no separate prepass.

out[b,s,h,c] = sum_j softmax_j(filters[b,s,h,:])[j] * x_pad[b, s+j-4, h*64+c]

v7 over v4-baseline: products in bf16 on both operands. x is cast f32->bf16
once per tile (ACT). The staggered softmax filters are evacuated from PSUM as
DUPLICATED PAIRS fn_s2[p, j, h, 2] (two ACT copies) so the DVE product's
filter operand has an innermost step-1 pair dim -> hardware auto-selects the
2x_1P perf mode (16-bit, step 1, 4B aligned) despite the c-broadcast, halving
DVE product time. HW-verified exact vs bf16 numpy (microbench).

Per-core, per main tile (TW=120 output tokens, 35 tiles):
  x_t  [128,768] f32 <- x rows [t0-4, t0+124)      (halo for the 9 taps)
  x_b  [128,768] bf16 (ACT cast)
  fr_t [128,108] f32 <- f rows [t0-8, t0+120)      (halo for the stagger)
  softmax in-tile: exp (ACT), reduce+recip (DVE), normalize -> bf16 [p,j,h]
    (Pool, transposing (h,j)->(j,h) via strided read).
  stagger via PE: for each tap j, matmul with lhsT = B[:, 16-j : 144-j]
    (column slice of one inline band matrix B[k,c]=[k==c-8], bf16) so
    fn_ps[k,j,h] = fn[t0+k-j,h,j] -- the filter value multiplying x row
    t0-4+k toward output row t0+k-j. Rows with no identity hit get 0.
  evac fn_s2 [128,9,12,2] bf16 <- fn_ps twice (ACT; strided pair writes).
  products: P[k,j,hc] = x_b[k,hc] * fn_s2[k,j,h,{0,1}] -- DVE takes taps
    [0,NDV) in one op with the pair AP; Pool (gpsimd) takes taps [NDV,9)
    with a plain c-broadcast AP. Separate dest tensors p_a/p_b.
  shift-sum: out[t0+m] = sum_j P[m+j,j] -- lhsT = ident[:, j:j+tw] column
    slices of one 128x128 bf16 identity, 18 accumulating PE matmuls into
    PSUM (exact: weights 0/1), in 512+256 column chunks.
  evac o_t (ACT copy), DMA out.
Zero padding at sequence edges via memset of x_t / fr_t edge rows
(exp(0)=1 keeps softmax finite; stagger zeros + x zeros keep PE NaN-free).
"""

import os
import sys

import numpy as np

for _p in ("/opt/trn_rl_repo",):
    if _p not in sys.path:
        sys.path.insert(0, _p)

B, S, D = 8, 4096, 768
H, HD, KS = 12, 64, 9
PAD = KS // 2  # 4
TW = 120  # output tokens per main tile
NT = (S + TW - 1) // TW  # 35 tiles; last covers 16 tokens

_CACHE = {}


def _build_program(loop_n: int | None = None):
    """loop_n=None: single-shot program (used by kernel()). loop_n=K: the
    whole kernel body runs K times under a tc.For_i hardware loop — one NEFF
    execution = K complete DRAM->DRAM kernel executions (bench)."""
    import concourse.bass as bass
    import concourse.tile as tile
    from concourse import mybir

    f32 = mybir.dt.float32
    bf16 = mybir.dt.bfloat16

    NDV = int(os.environ.get("LC_V7_NDV", "7"))  # taps on DVE; rest on Pool
    PLAIN = bool(os.environ.get("LC_V7_PLAIN"))  # no pair trick (A/B)
    F32P = bool(os.environ.get("LC_V7_F32"))  # f32 products fallback
    XCAST = os.environ.get("LC_V7_XCAST", "act")  # act|dve

    nc = bass.Bass()
    x_d = nc.dram_tensor("x", [S, D], f32, kind="ExternalInput")
    f_d = nc.dram_tensor("f", [S, H * KS], f32, kind="ExternalInput")
    o_d = nc.dram_tensor("o", [S, D], f32, kind="ExternalOutput")

    # ident[k,m] = [k==m]; shift-sum lhsT_j = ident[:, j : j+tw]
    id_np = np.eye(128, dtype=np.float32)
    id_d = nc.inline_tensor(id_np, name="ident")
    # band B[k,c] = [k == c-8]; stagger lhsT_j = B[:, 16-j : 144-j]
    b_np = np.zeros((128, 144), dtype=np.float32)
    for k in range(128):
        b_np[k, k + 8] = 1.0
    b_d = nc.inline_tensor(b_np, name="band")

    with tile.TileContext(nc) as tc:
        with (
            tc.tile_pool(name="singles", bufs=1) as singles,
            tc.tile_pool(name="fin", bufs=4) as fin,
            tc.tile_pool(name="xin", bufs=4) as xin,
            tc.tile_pool(name="xbp", bufs=3) as xbp,
            tc.tile_pool(name="sfx", bufs=4) as sfx,
            tc.tile_pool(name="fst", bufs=4) as fst,
            tc.tile_pool(name="prod", bufs=3) as prod,
            tc.tile_pool(name="prodb", bufs=3) as prodb,
            tc.tile_pool(name="outs", bufs=4) as outs,
            tc.tile_pool(name="psf", bufs=2, space="PSUM") as psf,
            tc.tile_pool(name="ps", bufs=3, space="PSUM") as ps,
        ):
            id_f32 = singles.tile([128, 128], f32)
            nc.sync.dma_start(out=id_f32, in_=id_d[:, :])
            id_sb = singles.tile([128, 128], bf16)
            nc.vector.tensor_copy(id_sb, id_f32)
            b_f32 = singles.tile([128, 144], f32)
            nc.sync.dma_start(out=b_f32, in_=b_d[:, :])
            b_sb = singles.tile([128, 144], bf16)
            nc.vector.tensor_copy(b_sb, b_f32)

            Copy = mybir.ActivationFunctionType.Copy
            Exp = mybir.ActivationFunctionType.Exp

            def _kernel_body():
                for t in range(NT):
                    t0 = TW * t
                    tw = min(TW, S - t0)  # valid out tokens (16 on last tile)
                    u0 = t0 - PAD  # first x row of this u-tile
                    f0 = t0 - 2 * PAD  # first f row (stagger halo)

                    x_t = xin.tile([128, D], f32, tag="x_t")
                    if t == 0:
                        nc.vector.memset(x_t[0:PAD, :], 0.0)
                        nc.sync.dma_start(
                            out=x_t[PAD:128, :], in_=x_d[0 : 128 - PAD, :]
                        )
                    elif u0 + 128 > S:
                        nv = S - u0
                        nc.vector.memset(x_t, 0.0)
                        nc.sync.dma_start(out=x_t[0:nv, :], in_=x_d[u0:S, :])
                    else:
                        nc.sync.dma_start(out=x_t, in_=x_d[u0 : u0 + 128, :])

                    fr_t = fin.tile([128, H * KS], f32, tag="fr_t")
                    if t == 0:
                        nc.gpsimd.memset(fr_t[0 : 2 * PAD, :], 0.0)
                        nc.scalar.dma_start(
                            out=fr_t[2 * PAD : 128, :], in_=f_d[0 : 128 - 2 * PAD, :]
                        )
                    elif f0 + 128 > S:
                        nv = S - f0
                        nc.gpsimd.memset(fr_t, 0.0)
                        nc.scalar.dma_start(out=fr_t[0:nv, :], in_=f_d[f0:S, :])
                    else:
                        nc.scalar.dma_start(out=fr_t, in_=f_d[f0 : f0 + 128, :])

                    # x cast to bf16 (skipped for f32 products)
                    if not F32P:
                        x_b = xbp.tile([128, D], bf16, tag="x_b")
                        if XCAST == "dve":
                            nc.vector.tensor_copy(x_b, x_t)
                        else:
                            nc.scalar.activation(x_b, x_t, Copy)
                    else:
                        x_b = x_t

                    e_t = sfx.tile([128, H * KS], f32, tag="e_t")
                    nc.scalar.activation(e_t, fr_t, Exp)
                    z_t = sfx.tile([128, H], f32, tag="z_t")
                    nc.vector.tensor_reduce(
                        out=z_t,
                        in_=e_t.rearrange("p (h j) -> p h j", j=KS),
                        axis=mybir.AxisListType.X,
                        op=mybir.AluOpType.add,
                    )
                    r_t = sfx.tile([128, H], f32, tag="r_t")
                    nc.vector.reciprocal(r_t, z_t)
                    fnb = sfx.tile([128, KS, H], bf16, tag="fnb")
                    nc.gpsimd.tensor_mul(
                        fnb,
                        e_t.rearrange("p (h j) -> p j h", j=KS),
                        r_t.unsqueeze(1).broadcast_to([128, KS, H]),
                    )

                    fn_ps = psf.tile([128, KS, H], f32, tag="fn_ps")
                    for j in range(KS):
                        nc.tensor.matmul(
                            fn_ps[:, j, :],
                            b_sb[:, 16 - j : 144 - j],
                            fnb[:, j, :],
                            start=True,
                            stop=True,
                        )

                    if F32P:
                        fn_s = fst.tile([128, KS, H], f32, tag="fn_s")
                        nc.scalar.activation(fn_s, fn_ps, Copy)
                    else:
                        # pairs: fn_s2[p,j,h,0] == fn_s2[p,j,h,1]
                        fn_s2 = fst.tile([128, KS, H, 2], bf16, tag="fn_s2")
                        nc.scalar.activation(fn_s2[:, :, :, 0], fn_ps, Copy)
                        nc.scalar.activation(fn_s2[:, :, :, 1], fn_ps, Copy)

                    # products -> p_a (DVE taps [0,NDV)), p_b (Pool taps)
                    NPOOL = KS - NDV
                    p_a = prod.tile([128, NDV, D], bf16, tag="p_a")
                    if NPOOL:
                        p_b = prodb.tile([128, NPOOL, D], bf16, tag="p_b")
                    if F32P:
                        x_hc = x_t.rearrange("p (h c) -> p h c", c=HD)
                        nc.vector.tensor_mul(
                            p_a.rearrange("p j (h c) -> p j h c", c=HD),
                            x_hc.unsqueeze(1).broadcast_to([128, NDV, H, HD]),
                            fn_s[:, 0:NDV, :]
                            .unsqueeze(3)
                            .broadcast_to([128, NDV, H, HD]),
                        )
                        if NPOOL:
                            nc.gpsimd.tensor_mul(
                                p_b.rearrange("p j (h c) -> p j h c", c=HD),
                                x_hc.unsqueeze(1).broadcast_to(
                                    [128, NPOOL, H, HD]
                                ),
                                fn_s[:, NDV:KS, :]
                                .unsqueeze(3)
                                .broadcast_to([128, NPOOL, H, HD]),
                            )
                    else:
                        if PLAIN:
                            x_hc = x_b.rearrange("p (h c) -> p h c", c=HD)
                            nc.vector.tensor_mul(
                                p_a.rearrange("p j (h c) -> p j h c", c=HD),
                                x_hc.unsqueeze(1).broadcast_to(
                                    [128, NDV, H, HD]
                                ),
                                fn_s2[:, 0:NDV, :, 0]
                                .unsqueeze(3)
                                .broadcast_to([128, NDV, H, HD]),
                            )
                        else:
                            C2 = HD // 2
                            x_v = x_b.rearrange(
                                "p (h c2 two) -> p h c2 two", h=H, c2=C2
                            )
                            nc.vector.tensor_mul(
                                p_a.rearrange(
                                    "p j (h c2 two) -> p j h c2 two", h=H, c2=C2
                                ),
                                x_v.unsqueeze(1).broadcast_to(
                                    [128, NDV, H, C2, 2]
                                ),
                                fn_s2[:, 0:NDV]
                                .unsqueeze(3)
                                .broadcast_to([128, NDV, H, C2, 2]),
                            )
                        if NPOOL:
                            x_hc = x_b.rearrange("p (h c) -> p h c", c=HD)
                            nc.gpsimd.tensor_mul(
                                p_b.rearrange("p j (h c) -> p j h c", c=HD),
                                x_hc.unsqueeze(1).broadcast_to(
                                    [128, NPOOL, H, HD]
                                ),
                                fn_s2[:, NDV:KS, :, 0]
                                .unsqueeze(3)
                                .broadcast_to([128, NPOOL, H, HD]),
                            )

                    o_ps = ps.tile([128, D], f32, tag="o_ps")
                    for j in range(KS):
                        lhsT = id_sb[:, j : j + tw]
                        rhs_t = p_a if j < NDV else p_b
                        jj = j if j < NDV else j - NDV
                        for n0, n1 in ((0, 512), (512, D)):
                            nc.tensor.matmul(
                                o_ps[0:tw, n0:n1],
                                lhsT,
                                rhs_t[:, jj, n0:n1],
                                start=(j == 0),
                                stop=(j == KS - 1),
                            )

                    o_t = outs.tile([128, D], f32, tag="o_t")
                    nc.scalar.activation(o_t[0:tw, :], o_ps[0:tw, :], Copy)
                    nc.sync.dma_start(out=o_d[t0 : t0 + tw, :], in_=o_t[0:tw, :])

            if loop_n is None:
                _kernel_body()
            else:
                with tc.For_i(0, loop_n):
                    _kernel_body()

    _split_hwdge_multi_waits(nc)
    return nc


def _split_hwdge_multi_waits(nc):
    """walrus's HWDGE DMA trigger (PSEUDO_DMA_DIRECT2D) rejects >1 sync wait
    on a DMACopy. Move all but one wait onto a NoOp inserted right before the
    DMA on the same (sequencer) engine — identical semantics, since the
    sequencer executes both in order before triggering the descriptor."""
    from concourse import mybir

    nsplit = 0
    for fn in nc.m.functions:
        for blk in fn.blocks:
            out = []
            for ins in blk.instructions:
                si = ins.sync_info
                if si is not None and len(si.on_wait) > 1:
                    for wi, w in enumerate(si.on_wait[:-1]):
                        nop = mybir.InstNoOp(
                            name=f"{ins.name}_waitsplit{wi}",
                            engine=ins.engine,
                            sync_info=mybir.SyncInfo(on_wait=[w], on_update=[]),
                        )
                        out.append(nop)
                    ins.sync_info = mybir.SyncInfo(
                        on_wait=list(si.on_wait[-1:]),
                        on_update=list(si.on_update),
                    )
                    nsplit += 1
                out.append(ins)
            blk.instructions = out
    if nsplit and os.environ.get("LC_DEBUG"):
        print(f"_split_hwdge_multi_waits: split {nsplit} DMAs")


def kernel(inputs: np.ndarray, filters: np.ndarray) -> np.ndarray:
    from concourse.bass_utils import run_bass_kernel_spmd

    if "nc" not in _CACHE:
        _CACHE["nc"] = _build_program()
    nc = _CACHE["nc"]

    inputs = np.ascontiguousarray(np.asarray(inputs, dtype=np.float32))
    filters = np.ascontiguousarray(np.asarray(filters, dtype=np.float32))

    in_maps = [{"x": inputs[c], "f": filters[c]} for c in range(B)]

    res = run_bass_kernel_spmd(nc, in_maps, core_ids=list(range(B)), trace=False)

    out = np.stack([res.results[c]["o"] for c in range(B)], axis=0)
    return out.reshape(B, S, H, HD)

def bench(
    inputs: np.ndarray, filters: np.ndarray, reps: int = 20, loop_n: int = 1000
) -> float:
    """Steady-state device benchmark. One NEFF launch executes the complete
    kernel (prepass + main pass, full DRAM->DRAM dataflow) ``loop_n`` times
    under a tc.For_i hardware loop; ``reps`` launches are timed back-to-back
    after a warm-up launch. Returns mean seconds per kernel execution —
    launch/transfer overheads are amortized over reps*loop_n executions."""
    import time

    import jax
    from jax.experimental.shard_map import shard_map
    from jax.sharding import Mesh, PartitionSpec

    import concourse.mybir as mybir
    from concourse import bass2jax

    key = f"nc_loop{loop_n}"
    if key not in _CACHE:
        _CACHE[key] = _build_program(loop_n=loop_n)
    nc = _CACHE[key]
    bass2jax.install_neuronx_cc_hook()

    part_name = nc.partition_id_tensor.name if nc.partition_id_tensor else None
    in_names, out_names, out_avals, zero_outs = [], [], [], []
    for alloc in nc.m.functions[0].allocations:
        if not isinstance(alloc, mybir.MemoryLocationSet):
            continue
        name = alloc.memorylocations[0].name
        if alloc.kind == "ExternalInput":
            if name != part_name:
                in_names.append(name)
        elif alloc.kind == "ExternalOutput":
            out_names.append(name)
            shape = tuple(alloc.tensor_shape)
            dtype = mybir.dt.np(alloc.dtype)
            out_avals.append(jax.core.ShapedArray(shape, dtype))
            zero_outs.append(np.zeros(shape, dtype))
    n_params = len(in_names)
    all_names = in_names + out_names
    if part_name is not None:
        all_names = all_names + [part_name]

    def _body(*args):
        operands = list(args)
        if part_name is not None:
            operands.append(bass2jax.partition_id_tensor())
        outs = bass2jax._bass_exec_p.bind(
            *operands,
            out_avals=tuple(out_avals),
            in_names=tuple(all_names),
            out_names=tuple(out_names),
            lowering_input_output_aliases=(),
            sim_require_finite=True,
            sim_require_nnan=True,
            nc=nc,
        )
        return tuple(outs)

    devices = jax.devices()[:B]
    mesh = Mesh(np.asarray(devices), ("core",))
    nin = n_params + len(out_names)
    fn = jax.jit(
        shard_map(
            _body,
            mesh=mesh,
            in_specs=(PartitionSpec("core"),) * nin,
            out_specs=(PartitionSpec("core"),) * len(out_names),
            check_rep=False,
        ),
        keep_unused=True,
    )
    per_core = {"x": inputs.astype(np.float32), "f": filters.astype(np.float32)}
    concat_in = [
        np.concatenate([per_core[n][c] for c in range(B)], axis=0) for n in in_names
    ]
    concat_zero = [
        np.zeros((B * z.shape[0], *z.shape[1:]), z.dtype) for z in zero_outs
    ]
    sharding = jax.sharding.NamedSharding(mesh, PartitionSpec("core"))
    dev_args = [jax.device_put(a, sharding) for a in concat_in + concat_zero]

    out = fn(*dev_args)  # compile + warm
    jax.block_until_ready(out)
    t0 = time.perf_counter()
    for _ in range(reps):
        out = fn(*dev_args)
    jax.block_until_ready(out)
    t1 = time.perf_counter()
    return (t1 - t0) / (reps * loop_n)


if __name__ == "__main__":
    rng = np.random.default_rng(0)
    x = rng.standard_normal((B, S, D), dtype=np.float32)
    f = rng.standard_normal((B, S, H * KS), dtype=np.float32)
    o = kernel(x, f)
    print(o.shape, o.dtype)


# revision 3
# speedup vs baseline: 1.4165x; 1.4038x over previous
"""ConvBert LightConv kernel v7 for Trainium2 (Bass/Tile), batch-parallel on 8
cores. Fully fused: no DRAM scratch, no separate prepass.

out[b,s,h,c] = sum_j softmax_j(filters[b,s,h,:])[j] * x_pad[b, s+j-4, h*64+c]

v7 over v4-baseline: products in bf16 on both operands. x is cast f32->bf16
once per tile (ACT). The staggered softmax filters are evacuated from PSUM as
DUPLICATED PAIRS fn_s2[p, j, h, 2] (two ACT copies) so the DVE product's
filter operand has an innermost step-1 pair dim -> hardware auto-selects the
2x_1P perf mode (16-bit, step 1, 4B aligned) despite the c-broadcast, halving
DVE product time. HW-verified exact vs bf16 numpy (microbench).

Per-core, per main tile (TW=120 output tokens, 35 tiles):
  x_t  [128,768] f32 <- x rows [t0-4, t0+124)      (halo for the 9 taps)
  x_b  [128,768] bf16 (ACT cast)
  fr_t [128,108] f32 <- f rows [t0-8, t0+120)      (halo for the stagger)
  softmax in-tile: exp (ACT), reduce+recip (DVE), normalize -> bf16 [p,j,h]
    (Pool, transposing (h,j)->(j,h) via strided read).
  stagger via PE: for each tap j, matmul with lhsT = B[:, 16-j : 144-j]
    (column slice of one inline band matrix B[k,c]=[k==c-8], bf16) so
    fn_ps[k,j,h] = fn[t0+k-j,h,j] -- the filter value multiplying x row
    t0-4+k toward output row t0+k-j. Rows with no identity hit get 0.
  evac fn_s2 [128,9,12,2] bf16 <- fn_ps twice (ACT; strided pair writes).
  products: P[k,j,hc] = x_b[k,hc] * fn_s2[k,j,h,{0,1}] -- DVE takes taps
    [0,NDV) in one op with the pair AP; Pool (gpsimd) takes taps [NDV,9)
    with a plain c-broadcast AP. Separate dest tensors p_a/p_b.
  shift-sum: out[t0+m] = sum_j P[m+j,j] -- lhsT = ident[:, j:j+tw] column
    slices of one 128x128 bf16 identity, 18 accumulating PE matmuls into
    PSUM (exact: weights 0/1), in 512+256 column chunks.
  evac o_t (ACT copy), DMA out.
Zero padding at sequence edges via memset of x_t / fr_t edge rows
(exp(0)=1 keeps softmax finite; stagger zeros + x zeros keep PE NaN-free).
"""

import os
import sys

import numpy as np

for _p in ("/opt/trn_rl_repo",):
    if _p not in sys.path:
        sys.path.insert(0, _p)

B, S, D = 8, 4096, 768
H, HD, KS = 12, 64, 9
PAD = KS // 2  # 4
TW = 120  # output tokens per main tile
NT = (S + TW - 1) // TW  # 35 tiles; last covers 16 tokens

_CACHE = {}


def _build_program(loop_n: int | None = None):
    """loop_n=None: single-shot program (used by kernel()). loop_n=K: the
    whole kernel body runs K times under a tc.For_i hardware loop — one NEFF
    execution = K complete DRAM->DRAM kernel executions (bench)."""
    import concourse.bass as bass
    import concourse.tile as tile
    from concourse import mybir

    f32 = mybir.dt.float32
    bf16 = mybir.dt.bfloat16

    NDV = int(os.environ.get("LC_V7_NDV", "7"))  # taps on DVE; rest on Pool
    PLAIN = bool(os.environ.get("LC_V7_PLAIN"))  # no pair trick (A/B)
    F32P = bool(os.environ.get("LC_V7_F32"))  # f32 products fallback
    XCAST = os.environ.get("LC_V7_XCAST", "act")  # act|dve

    nc = bass.Bass()
    x_d = nc.dram_tensor("x", [S, D], f32, kind="ExternalInput")
    f_d = nc.dram_tensor("f", [S, H * KS], f32, kind="ExternalInput")
    o_d = nc.dram_tensor("o", [S, D], f32, kind="ExternalOutput")

    # ident[k,m] = [k==m] padded to 136 cols; backhalf lhsT_j = ident[:, j:j+128]
    # (128-wide bf16 weights -> compiler enables FWL: ~2x faster LDWEIGHTS)
    id_np = np.zeros((128, 136), dtype=np.float32)
    id_np[:, :128] = np.eye(128, dtype=np.float32)
    id_d = nc.inline_tensor(id_np, name="ident")
    # band B[k,c] = [k == c-8]; stagger lhsT_j = B[:, 16-j : 144-j]
    b_np = np.zeros((128, 144), dtype=np.float32)
    for k in range(128):
        b_np[k, k + 8] = 1.0
    b_d = nc.inline_tensor(b_np, name="band")

    with tile.TileContext(nc) as tc:
        with (
            tc.tile_pool(name="singles", bufs=1) as singles,
            tc.tile_pool(name="fin", bufs=4) as fin,
            tc.tile_pool(name="xin", bufs=4) as xin,
            tc.tile_pool(name="xbp", bufs=3) as xbp,
            tc.tile_pool(name="sfx", bufs=4) as sfx,
            tc.tile_pool(name="fst", bufs=4) as fst,
            tc.tile_pool(name="prod", bufs=3) as prod,
            tc.tile_pool(name="prodb", bufs=3) as prodb,
            tc.tile_pool(name="outs", bufs=4) as outs,
            tc.tile_pool(name="psf", bufs=2, space="PSUM") as psf,
            tc.tile_pool(name="ps", bufs=3, space="PSUM") as ps,
        ):
            id_f32 = singles.tile([128, 136], f32)
            nc.sync.dma_start(out=id_f32, in_=id_d[:, :])
            id_sb = singles.tile([128, 136], bf16)
            nc.vector.tensor_copy(id_sb, id_f32)
            b_f32 = singles.tile([128, 144], f32)
            nc.sync.dma_start(out=b_f32, in_=b_d[:, :])
            b_sb = singles.tile([128, 144], bf16)
            nc.vector.tensor_copy(b_sb, b_f32)

            Copy = mybir.ActivationFunctionType.Copy
            Exp = mybir.ActivationFunctionType.Exp

            def _kernel_body():
                for t in range(NT):
                    t0 = TW * t
                    tw = min(TW, S - t0)  # valid out tokens (16 on last tile)
                    u0 = t0 - PAD  # first x row of this u-tile
                    f0 = t0 - 2 * PAD  # first f row (stagger halo)

                    x_t = xin.tile([128, D], f32, tag="x_t")
                    if t == 0:
                        nc.vector.memset(x_t[0:PAD, :], 0.0)
                        nc.sync.dma_start(
                            out=x_t[PAD:128, :], in_=x_d[0 : 128 - PAD, :]
                        )
                    elif u0 + 128 > S:
                        nv = S - u0
                        nc.vector.memset(x_t, 0.0)
                        nc.sync.dma_start(out=x_t[0:nv, :], in_=x_d[u0:S, :])
                    else:
                        nc.sync.dma_start(out=x_t, in_=x_d[u0 : u0 + 128, :])

                    fr_t = fin.tile([128, H * KS], f32, tag="fr_t")
                    if t == 0:
                        nc.gpsimd.memset(fr_t[0 : 2 * PAD, :], 0.0)
                        nc.scalar.dma_start(
                            out=fr_t[2 * PAD : 128, :], in_=f_d[0 : 128 - 2 * PAD, :]
                        )
                    elif f0 + 128 > S:
                        nv = S - f0
                        nc.gpsimd.memset(fr_t, 0.0)
                        nc.scalar.dma_start(out=fr_t[0:nv, :], in_=f_d[f0:S, :])
                    else:
                        nc.scalar.dma_start(out=fr_t, in_=f_d[f0 : f0 + 128, :])

                    # x cast to bf16 (skipped for f32 products)
                    if not F32P:
                        x_b = xbp.tile([128, D], bf16, tag="x_b")
                        if XCAST == "dve":
                            nc.vector.tensor_copy(x_b, x_t)
                        else:
                            nc.scalar.activation(x_b, x_t, Copy)
                    else:
                        x_b = x_t

                    e_t = sfx.tile([128, H * KS], f32, tag="e_t")
                    nc.scalar.activation(e_t, fr_t, Exp)
                    z_t = sfx.tile([128, H], f32, tag="z_t")
                    nc.vector.tensor_reduce(
                        out=z_t,
                        in_=e_t.rearrange("p (h j) -> p h j", j=KS),
                        axis=mybir.AxisListType.X,
                        op=mybir.AluOpType.add,
                    )
                    r_t = sfx.tile([128, H], f32, tag="r_t")
                    nc.vector.reciprocal(r_t, z_t)
                    fnb = sfx.tile([128, KS, H], bf16, tag="fnb")
                    nc.gpsimd.tensor_mul(
                        fnb,
                        e_t.rearrange("p (h j) -> p j h", j=KS),
                        r_t.unsqueeze(1).broadcast_to([128, KS, H]),
                    )

                    fn_ps = psf.tile([128, KS, H], f32, tag="fn_ps")
                    for j in range(KS):
                        nc.tensor.matmul(
                            fn_ps[:, j, :],
                            b_sb[:, 16 - j : 144 - j],
                            fnb[:, j, :],
                            start=True,
                            stop=True,
                        )

                    if F32P:
                        fn_s = fst.tile([128, KS, H], f32, tag="fn_s")
                        nc.scalar.activation(fn_s, fn_ps, Copy)
                    else:
                        # pairs: fn_s2[p,j,h,0] == fn_s2[p,j,h,1]
                        fn_s2 = fst.tile([128, KS, H, 2], bf16, tag="fn_s2")
                        nc.scalar.activation(fn_s2[:, :, :, 0], fn_ps, Copy)
                        nc.scalar.activation(fn_s2[:, :, :, 1], fn_ps, Copy)

                    # products -> p_a (DVE taps [0,NDV)), p_b (Pool taps)
                    NPOOL = KS - NDV
                    p_a = prod.tile([128, NDV, D], bf16, tag="p_a")
                    if NPOOL:
                        p_b = prodb.tile([128, NPOOL, D], bf16, tag="p_b")
                    if F32P:
                        x_hc = x_t.rearrange("p (h c) -> p h c", c=HD)
                        nc.vector.tensor_mul(
                            p_a.rearrange("p j (h c) -> p j h c", c=HD),
                            x_hc.unsqueeze(1).broadcast_to([128, NDV, H, HD]),
                            fn_s[:, 0:NDV, :]
                            .unsqueeze(3)
                            .broadcast_to([128, NDV, H, HD]),
                        )
                        if NPOOL:
                            nc.gpsimd.tensor_mul(
                                p_b.rearrange("p j (h c) -> p j h c", c=HD),
                                x_hc.unsqueeze(1).broadcast_to(
                                    [128, NPOOL, H, HD]
                                ),
                                fn_s[:, NDV:KS, :]
                                .unsqueeze(3)
                                .broadcast_to([128, NPOOL, H, HD]),
                            )
                    else:
                        if PLAIN:
                            x_hc = x_b.rearrange("p (h c) -> p h c", c=HD)
                            nc.vector.tensor_mul(
                                p_a.rearrange("p j (h c) -> p j h c", c=HD),
                                x_hc.unsqueeze(1).broadcast_to(
                                    [128, NDV, H, HD]
                                ),
                                fn_s2[:, 0:NDV, :, 0]
                                .unsqueeze(3)
                                .broadcast_to([128, NDV, H, HD]),
                            )
                        else:
                            C2 = HD // 2
                            x_v = x_b.rearrange(
                                "p (h c2 two) -> p h c2 two", h=H, c2=C2
                            )
                            nc.vector.tensor_mul(
                                p_a.rearrange(
                                    "p j (h c2 two) -> p j h c2 two", h=H, c2=C2
                                ),
                                x_v.unsqueeze(1).broadcast_to(
                                    [128, NDV, H, C2, 2]
                                ),
                                fn_s2[:, 0:NDV]
                                .unsqueeze(3)
                                .broadcast_to([128, NDV, H, C2, 2]),
                            )
                        if NPOOL:
                            x_hc = x_b.rearrange("p (h c) -> p h c", c=HD)
                            nc.gpsimd.tensor_mul(
                                p_b.rearrange("p j (h c) -> p j h c", c=HD),
                                x_hc.unsqueeze(1).broadcast_to(
                                    [128, NPOOL, H, HD]
                                ),
                                fn_s2[:, NDV:KS, :, 0]
                                .unsqueeze(3)
                                .broadcast_to([128, NPOOL, H, HD]),
                            )

                    o_ps = ps.tile([128, D], f32, tag="o_ps")
                    PADW = not os.environ.get("LC_V8_NOPAD")
                    for j in range(KS):
                        lhsT = id_sb[:, j : j + 128] if PADW else id_sb[:, j : j + tw]
                        mw = 128 if PADW else tw
                        rhs_t = p_a if j < NDV else p_b
                        jj = j if j < NDV else j - NDV
                        for n0, n1 in ((0, 512), (512, D)):
                            nc.tensor.matmul(
                                o_ps[0:mw, n0:n1],
                                lhsT,
                                rhs_t[:, jj, n0:n1],
                                start=(j == 0),
                                stop=(j == KS - 1),
                            )

                    o_t = outs.tile([128, D], f32, tag="o_t")
                    nc.scalar.activation(o_t[0:tw, :], o_ps[0:tw, :], Copy)
                    nc.sync.dma_start(out=o_d[t0 : t0 + tw, :], in_=o_t[0:tw, :])

            if loop_n is None:
                _kernel_body()
            else:
                with tc.For_i(0, loop_n):
                    _kernel_body()

    _split_hwdge_multi_waits(nc)
    return nc


def _split_hwdge_multi_waits(nc):
    """walrus's HWDGE DMA trigger (PSEUDO_DMA_DIRECT2D) rejects >1 sync wait
    on a DMACopy. Move all but one wait onto a NoOp inserted right before the
    DMA on the same (sequencer) engine — identical semantics, since the
    sequencer executes both in order before triggering the descriptor."""
    from concourse import mybir

    nsplit = 0
    for fn in nc.m.functions:
        for blk in fn.blocks:
            out = []
            for ins in blk.instructions:
                si = ins.sync_info
                if si is not None and len(si.on_wait) > 1:
                    for wi, w in enumerate(si.on_wait[:-1]):
                        nop = mybir.InstNoOp(
                            name=f"{ins.name}_waitsplit{wi}",
                            engine=ins.engine,
                            sync_info=mybir.SyncInfo(on_wait=[w], on_update=[]),
                        )
                        out.append(nop)
                    ins.sync_info = mybir.SyncInfo(
                        on_wait=list(si.on_wait[-1:]),
                        on_update=list(si.on_update),
                    )
                    nsplit += 1
                out.append(ins)
            blk.instructions = out
    if nsplit and os.environ.get("LC_DEBUG"):
        print(f"_split_hwdge_multi_waits: split {nsplit} DMAs")


def kernel(inputs: np.ndarray, filters: np.ndarray) -> np.ndarray:
    from concourse.bass_utils import run_bass_kernel_spmd

    if "nc" not in _CACHE:
        _CACHE["nc"] = _build_program()
    nc = _CACHE["nc"]

    inputs = np.ascontiguousarray(np.asarray(inputs, dtype=np.float32))
    filters = np.ascontiguousarray(np.asarray(filters, dtype=np.float32))

    in_maps = [{"x": inputs[c], "f": filters[c]} for c in range(B)]

    res = run_bass_kernel_spmd(nc, in_maps, core_ids=list(range(B)), trace=False)

    out = np.stack([res.results[c]["o"] for c in range(B)], axis=0)
    return out.reshape(B, S, H, HD)

def bench(
    inputs: np.ndarray, filters: np.ndarray, reps: int = 20, loop_n: int = 1000
) -> float:
    """Steady-state device benchmark. One NEFF launch executes the complete
    kernel (prepass + main pass, full DRAM->DRAM dataflow) ``loop_n`` times
    under a tc.For_i hardware loop; ``reps`` launches are timed back-to-back
    after a warm-up launch. Returns mean seconds per kernel execution —
    launch/transfer overheads are amortized over reps*loop_n executions."""
    import time

    import jax
    from jax.experimental.shard_map import shard_map
    from jax.sharding import Mesh, PartitionSpec

    import concourse.mybir as mybir
    from concourse import bass2jax

    key = f"nc_loop{loop_n}"
    if key not in _CACHE:
        _CACHE[key] = _build_program(loop_n=loop_n)
    nc = _CACHE[key]
    bass2jax.install_neuronx_cc_hook()

    part_name = nc.partition_id_tensor.name if nc.partition_id_tensor else None
    in_names, out_names, out_avals, zero_outs = [], [], [], []
    for alloc in nc.m.functions[0].allocations:
        if not isinstance(alloc, mybir.MemoryLocationSet):
            continue
        name = alloc.memorylocations[0].name
        if alloc.kind == "ExternalInput":
            if name != part_name:
                in_names.append(name)
        elif alloc.kind == "ExternalOutput":
            out_names.append(name)
            shape = tuple(alloc.tensor_shape)
            dtype = mybir.dt.np(alloc.dtype)
            out_avals.append(jax.core.ShapedArray(shape, dtype))
            zero_outs.append(np.zeros(shape, dtype))
    n_params = len(in_names)
    all_names = in_names + out_names
    if part_name is not None:
        all_names = all_names + [part_name]

    def _body(*args):
        operands = list(args)
        if part_name is not None:
            operands.append(bass2jax.partition_id_tensor())
        outs = bass2jax._bass_exec_p.bind(
            *operands,
            out_avals=tuple(out_avals),
            in_names=tuple(all_names),
            out_names=tuple(out_names),
            lowering_input_output_aliases=(),
            sim_require_finite=True,
            sim_require_nnan=True,
            nc=nc,
        )
        return tuple(outs)

    devices = jax.devices()[:B]
    mesh = Mesh(np.asarray(devices), ("core",))
    nin = n_params + len(out_names)
    fn = jax.jit(
        shard_map(
            _body,
            mesh=mesh,
            in_specs=(PartitionSpec("core"),) * nin,
            out_specs=(PartitionSpec("core"),) * len(out_names),
            check_rep=False,
        ),
        keep_unused=True,
    )
    per_core = {"x": inputs.astype(np.float32), "f": filters.astype(np.float32)}
    concat_in = [
        np.concatenate([per_core[n][c] for c in range(B)], axis=0) for n in in_names
    ]
    concat_zero = [
        np.zeros((B * z.shape[0], *z.shape[1:]), z.dtype) for z in zero_outs
    ]
    sharding = jax.sharding.NamedSharding(mesh, PartitionSpec("core"))
    dev_args = [jax.device_put(a, sharding) for a in concat_in + concat_zero]

    out = fn(*dev_args)  # compile + warm
    jax.block_until_ready(out)
    t0 = time.perf_counter()
    for _ in range(reps):
        out = fn(*dev_args)
    jax.block_until_ready(out)
    t1 = time.perf_counter()
    return (t1 - t0) / (reps * loop_n)


if __name__ == "__main__":
    rng = np.random.default_rng(0)
    x = rng.standard_normal((B, S, D), dtype=np.float32)
    f = rng.standard_normal((B, S, H * KS), dtype=np.float32)
    o = kernel(x, f)
    print(o.shape, o.dtype)
